# revision 2
# baseline (speedup 1.0000x reference)
"""Detection-loss kernel for Trainium2 (8 NeuronCores, data-parallel over batch).

Reference computes: scatter 64 targets/image into a [B,C,H,W] map + mask,
then masked SmoothL1(preds, map).sum() / num_objects.

The mask is nonzero at <= B*T positions, so the loss only depends on preds
at those positions.  Each core *gathers* preds at its images' (gy,gx) cells
via one indirect DMA (256 descriptors x 28B, channels-last relayout so one
descriptor moves all 7 channels), computes the SmoothL1 partials, and DMAs
a [128,1] per-partition partial column out.  Host sums 8x128 partials.

Everything derivable from the tiny `targets` tensor (14KB) is precomputed
on host: the flat gather offsets, and the last-writer-wins collision mask
(win) matching jax scatter semantics.  num_objects (= count of unique cells
per image) is likewise exact on host.  The win mask folds into the device
chain via mw = min(|d|, win): for win in {0,1},
  smoothl1(d)*win = mw*|d| - 0.5*mw^2.

Sharding layout per core (4 images, 2 groups of 128 targets on partitions):
  partition p in [0,128), group g in {0,1}:
    image j = g*2 + p//64 (local), target t = p%64, channel c in [0,7)
  flat gather offset = (gy*W + gx)*C + j*C*H*W, gy/gx = floor(coord * 5.0).
"""

import numpy as np

B, C, H, W = 32, 7, 400, 400
T = 64
NCORES = 8
BLOC = B // NCORES          # 4 images per core
HW = H * W                  # 160000
CHW = C * HW                # 1120000
NELEM = BLOC * CHW          # 4480000 elements per core
NG = BLOC * T // 128        # 2 groups of 128 targets
P = 128
GC = NG * C                 # 14 value columns

_cached = {}
TRACE = False  # set True (e.g. from test.py) to capture an NTFF profile


def _build_nc():
    import concourse.bacc as bacc
    import concourse.bass as bass
    import concourse.tile as tile
    import concourse.mybir as mybir

    f32 = mybir.dt.float32
    i32 = mybir.dt.int32
    OP = mybir.AluOpType
    AX = mybir.AxisListType

    nc = bacc.Bacc(
        "TRN2",
        target_bir_lowering=False,
        debug=False,
        enable_asserts=False,
        num_devices=NCORES,
    )

    preds_flat = nc.dram_tensor("preds_flat", [NELEM, 1], f32, kind="ExternalInput")
    aux_i = nc.dram_tensor("aux_i", [P, NG], i32, kind="ExternalInput")
    # aux_f: [tvals (14) | winx (14)]
    aux_f = nc.dram_tensor("aux_f", [P, 2 * GC], f32, kind="ExternalInput")
    out_d = nc.dram_tensor("out", [P, 1], f32, kind="ExternalOutput")

    with tile.TileContext(nc) as tc:
        with tc.tile_pool(name="sbuf", bufs=1) as sb:
            oi = sb.tile([P, NG], i32)
            nc.sync.dma_start(oi[:], aux_i[:, :])
            xf = sb.tile([P, 2 * GC], f32)
            nc.sync.dma_start(xf[:], aux_f[:, :])
            tv = xf[:, 0:GC]
            winx = xf[:, GC : 2 * GC]

            # single gather: 256 descriptors (2 per partition), 28B each
            gat = sb.tile([P, GC], f32)
            nc.gpsimd.indirect_dma_start(
                out=gat[:, :],
                out_offset=None,
                in_=preds_flat[:, :],
                in_offset=bass.IndirectOffsetOnAxis(ap=oi[:, :], axis=0),
            )

            # smoothl1(d)*win = mw*|d| - 0.5*mw^2 with mw = min(|d|, win)
            d = sb.tile([P, GC], f32)
            nc.vector.tensor_sub(d[:], gat[:], tv)
            ad = sb.tile([P, GC], f32)
            nc.vector.scalar_tensor_tensor(ad[:], d[:], -1.0, d[:], OP.mult, OP.max)
            mw = sb.tile([P, GC], f32)
            nc.vector.tensor_tensor(mw[:], ad[:], winx, OP.min)
            t2 = sb.tile([P, GC], f32)
            nc.vector.scalar_tensor_tensor(t2[:], mw[:], -0.5, ad[:], OP.mult, OP.add)
            le = sb.tile([P, GC], f32)
            nc.vector.tensor_mul(le[:], mw[:], t2[:])
            red = sb.tile([P, 1], f32)
            nc.vector.reduce_sum(red[:], le[:], axis=AX.X)
            nc.sync.dma_start(out_d[:, :], red[:])

    nc.compile()
    return nc


def _get_nc():
    if "nc" not in _cached:
        _cached["nc"] = _build_nc()
    return _cached["nc"]


def _host_prep(targets):
    """Grid cells, last-writer-wins mask, and num_objects from targets only."""
    gx = np.clip(np.floor(targets[:, :, 0] * np.float32(5.0)), 0, W - 1).astype(
        np.int64
    )
    gy = np.clip(np.floor(targets[:, :, 1] * np.float32(5.0)), 0, H - 1).astype(
        np.int64
    )
    cell = gy * W + gx  # [B,T]
    # win[b,t] = 1 iff t is the last target of image b hitting cell[b,t]
    win = np.zeros((B, T), np.float32)
    for b in range(B):
        last = {}
        for t in range(T):
            last[cell[b, t]] = t
        for t in last.values():
            win[b, t] = 1.0
    num = float(win.sum())
    return cell, win, num


def _regroup(x):
    """[4,T] per-image array -> [128,2] (partition p, group g) layout."""
    return np.ascontiguousarray(
        x.reshape(NG, 2, T).transpose(1, 2, 0).reshape(P, NG)
    )


def _make_in_maps(preds, targets):
    cell, win, num = _host_prep(targets)

    # channels-last relayout so each target's 7 channels are one contiguous
    # 28B indirect-DMA row
    preds_t = np.ascontiguousarray(preds.transpose(0, 2, 3, 1))

    jbase = (np.arange(BLOC, dtype=np.int64) * CHW)[:, None]  # [4,1]

    in_maps = []
    for k in range(NCORES):
        pshard = preds_t[k * BLOC : (k + 1) * BLOC].reshape(NELEM, 1)
        tshard = targets[k * BLOC : (k + 1) * BLOC]  # [4, 64, 7]
        # tvals[p, g*7+c] = tshard[g*2 + p//64, p%64, c]
        tvals = tshard.reshape(NG, 2, T, C).transpose(1, 2, 0, 3).reshape(P, GC)
        offs = _regroup(cell[k * BLOC : (k + 1) * BLOC] * C + jbase).astype(np.int32)
        winx = np.repeat(_regroup(win[k * BLOC : (k + 1) * BLOC]), C, axis=1)
        aux_f = np.ascontiguousarray(np.hstack([tvals, winx]).astype(np.float32))
        in_maps.append({"preds_flat": pshard, "aux_i": offs, "aux_f": aux_f})
    return in_maps, num


def kernel(preds, targets):
    from concourse.bass_utils import run_bass_kernel_spmd

    preds = np.ascontiguousarray(np.asarray(preds), dtype=np.float32)
    targets = np.ascontiguousarray(np.asarray(targets), dtype=np.float32)
    assert preds.shape == (B, C, H, W) and targets.shape == (B, T, C)

    nc = _get_nc()
    in_maps, num = _make_in_maps(preds, targets)
    res = run_bass_kernel_spmd(nc, in_maps, list(range(NCORES)), trace=TRACE)
    _cached["last_results"] = res

    lsum = np.float32(0.0)
    for k in range(NCORES):
        part = res.results[k]["out"].reshape(P)
        lsum = np.float32(lsum + np.float32(part.sum(dtype=np.float64)))
    loss = np.float32(lsum / np.float32(np.float32(num) + np.float32(1e-6)))
    return loss, np.float32(num)


# revision 5
# speedup vs baseline: 1.3038x; 1.3038x over previous
"""Detection-loss kernel for Trainium2 (8 NeuronCores, data-parallel over batch).

Reference computes: scatter 64 targets/image into a [B,C,H,W] map + mask,
then masked SmoothL1(preds, map).sum() / num_objects.

The mask is nonzero at <= B*T positions, so the loss only depends on preds
at those positions.  Each core *gathers* preds at its images' (gy,gx) cells
via one indirect DMA (256 descriptors), computes the SmoothL1 partials, and
reduces to a single scalar on-device (free-axis DVE reduce via the identity
smoothl1(d)*win = mw*|d| - 0.5*mw^2, mw = min(|d|, win), then a ones-vector
matmul for the partition axis).  One 4B descriptor writes the result out.

Descriptors are padded to 8 elements (32B) so each lands on its own 32B AXI
beat: two concurrent 28B writes to the same partition row would share the
beat at byte 28..32 and race on the read-modify-write.  preds_flat gets 8
elements of zero padding so the pad element of the last cell stays in
bounds.  The pad column is killed by win=0 in the aux data.

Everything derivable from the tiny `targets` tensor (14KB) is precomputed
on host: flat gather offsets, the last-writer-wins collision mask (win)
matching jax scatter semantics, and num_objects (count of unique cells per
image, exact).

Sharding layout per core (4 images, 2 groups of 128 targets on partitions):
  partition p in [0,128), group g in {0,1}:
    image j = g*2 + p//64 (local), target t = p%64, channel c in [0,7)
  flat gather offset = (gy*W + gx)*C + j*C*H*W, gy/gx = floor(coord * 5.0)
  (channels-last host relayout of preds so channels are contiguous).
"""

import numpy as np

B, C, H, W = 32, 7, 400, 400
T = 64
NCORES = 8
BLOC = B // NCORES          # 4 images per core
HW = H * W                  # 160000
CHW = C * HW                # 1120000
NELEM = BLOC * CHW          # 4480000 elements per core
PAD = 8                     # zero padding so 8-elem descriptors stay in bounds
NG = BLOC * T // 128        # 2 groups of 128 targets
P = 128
SLOT = 8                    # descriptor width in elements (32B aligned)
GC = NG * SLOT              # 16 value columns

_cached = {}
TRACE = False  # set True (e.g. from test.py) to capture an NTFF profile


def _build_nc():
    import concourse.bacc as bacc
    import concourse.bass as bass
    import concourse.tile as tile
    import concourse.mybir as mybir

    f32 = mybir.dt.float32
    i32 = mybir.dt.int32
    OP = mybir.AluOpType
    AX = mybir.AxisListType

    nc = bacc.Bacc(
        "TRN2",
        target_bir_lowering=False,
        debug=False,
        enable_asserts=False,
        num_devices=NCORES,
    )

    preds_flat = nc.dram_tensor(
        "preds_flat", [NELEM + PAD, 1], f32, kind="ExternalInput"
    )
    aux_i = nc.dram_tensor("aux_i", [P, NG], i32, kind="ExternalInput")
    # aux_f: [tvp (16) | winxp (16)], slot-padded
    aux_f = nc.dram_tensor("aux_f", [P, 2 * GC], f32, kind="ExternalInput")
    out_d = nc.dram_tensor("out", [1, 1], f32, kind="ExternalOutput")

    with tile.TileContext(nc) as tc:
        with (
            tc.tile_pool(name="sbuf", bufs=1) as sb,
            tc.tile_pool(name="psum", bufs=1, space="PSUM") as pp,
        ):
            oi = sb.tile([P, NG], i32)
            nc.sync.dma_start(oi[:], aux_i[:, :])
            xf = sb.tile([P, 2 * GC], f32)
            nc.sync.dma_start(xf[:], aux_f[:, :])
            tv = xf[:, 0:GC]
            winx = xf[:, GC : 2 * GC]

            ones = sb.tile([P, 1], f32)
            nc.vector.memset(ones[:], 1.0)

            # single gather: 256 descriptors (2 per partition), 32B each
            gat = sb.tile([P, GC], f32)
            nc.gpsimd.indirect_dma_start(
                out=gat[:, :],
                out_offset=None,
                in_=preds_flat[:, :],
                in_offset=bass.IndirectOffsetOnAxis(ap=oi[:, :], axis=0),
            )

            # smoothl1(d)*win = mw*|d| - 0.5*mw^2 with mw = min(|d|, win)
            d = sb.tile([P, GC], f32)
            nc.vector.tensor_sub(d[:], gat[:], tv)
            ad = sb.tile([P, GC], f32)
            nc.vector.scalar_tensor_tensor(ad[:], d[:], -1.0, d[:], OP.mult, OP.max)
            mw = sb.tile([P, GC], f32)
            nc.vector.tensor_tensor(mw[:], ad[:], winx, OP.min)
            su = sb.tile([P, GC], f32)
            nc.vector.scalar_tensor_tensor(su[:], mw[:], -0.5, ad[:], OP.mult, OP.add)
            le = sb.tile([P, GC], f32)
            nc.vector.tensor_mul(le[:], mw[:], su[:])

            # partition reduction on the idle PE: [1,16] = ones^T @ le
            ps = pp.tile([1, GC], f32)
            nc.tensor.matmul(ps[:], ones[:], le[:], start=True, stop=True)
            red = sb.tile([1, 1], f32)
            nc.vector.reduce_sum(red[:], ps[:], axis=AX.X)
            nc.sync.dma_start(out_d[:, :], red[:])

    nc.compile()
    return nc


def _get_nc():
    if "nc" not in _cached:
        _cached["nc"] = _build_nc()
    return _cached["nc"]


def _host_prep(targets):
    """Grid cells, last-writer-wins mask, and num_objects from targets only."""
    gx = np.clip(np.floor(targets[:, :, 0] * np.float32(5.0)), 0, W - 1).astype(
        np.int64
    )
    gy = np.clip(np.floor(targets[:, :, 1] * np.float32(5.0)), 0, H - 1).astype(
        np.int64
    )
    cell = gy * W + gx  # [B,T]
    # win[b,t] = 1 iff t is the last target of image b hitting cell[b,t]
    win = np.zeros((B, T), np.float32)
    for b in range(B):
        last = {}
        for t in range(T):
            last[cell[b, t]] = t
        for t in last.values():
            win[b, t] = 1.0
    num = float(win.sum())
    return cell, win, num


def _regroup(x):
    """[4,T] per-image array -> [128,2] (partition p, group g) layout."""
    return np.ascontiguousarray(
        x.reshape(NG, 2, T).transpose(1, 2, 0).reshape(P, NG)
    )


def _make_in_maps(preds, targets):
    cell, win, num = _host_prep(targets)

    # channels-last relayout so each target's 7 channels are one contiguous
    # indirect-DMA row
    preds_t = np.ascontiguousarray(preds.transpose(0, 2, 3, 1))

    jbase = (np.arange(BLOC, dtype=np.int64) * CHW)[:, None]  # [4,1]
    zpad = np.zeros((PAD, 1), np.float32)

    in_maps = []
    for k in range(NCORES):
        pshard = np.concatenate(
            [preds_t[k * BLOC : (k + 1) * BLOC].reshape(NELEM, 1), zpad]
        )
        tshard = targets[k * BLOC : (k + 1) * BLOC]  # [4, 64, 7]
        # tvp[p, g*8+c] = tshard[g*2 + p//64, p%64, c] (c<7), 0 at c=7
        tpad = np.zeros((BLOC, T, SLOT), np.float32)
        tpad[:, :, :C] = tshard
        tvp = tpad.reshape(NG, 2, T, SLOT).transpose(1, 2, 0, 3).reshape(P, GC)
        wpg = _regroup(win[k * BLOC : (k + 1) * BLOC])  # [128,2]
        winxp = np.zeros((P, GC), np.float32)
        winxp[:, 0:C] = wpg[:, 0:1]
        winxp[:, SLOT : SLOT + C] = wpg[:, 1:2]
        offs = _regroup(cell[k * BLOC : (k + 1) * BLOC] * C + jbase).astype(np.int32)
        aux_f = np.ascontiguousarray(np.hstack([tvp, winxp]).astype(np.float32))
        in_maps.append({"preds_flat": pshard, "aux_i": offs, "aux_f": aux_f})
    return in_maps, num


def kernel(preds, targets):
    from concourse.bass_utils import run_bass_kernel_spmd

    preds = np.ascontiguousarray(np.asarray(preds), dtype=np.float32)
    targets = np.ascontiguousarray(np.asarray(targets), dtype=np.float32)
    assert preds.shape == (B, C, H, W) and targets.shape == (B, T, C)

    nc = _get_nc()
    in_maps, num = _make_in_maps(preds, targets)
    res = run_bass_kernel_spmd(nc, in_maps, list(range(NCORES)), trace=TRACE)
    _cached["last_results"] = res

    lsum = np.float32(0.0)
    for k in range(NCORES):
        lsum = np.float32(lsum + np.float32(res.results[k]["out"].reshape(1)[0]))
    loss = np.float32(lsum / np.float32(np.float32(num) + np.float32(1e-6)))
    return loss, np.float32(num)


# revision 6
# speedup vs baseline: 1.3276x; 1.0182x over previous
"""Detection-loss kernel for Trainium2 — raw Bass (no TileContext).

Same algorithm as v3 (one 256-descriptor indirect gather of 32B slots,
5-op DVE smoothl1 chain with the collision mask folded in via
mw = min(|d|, win), ones-matmul partition reduce, 4B output) but with
hand-placed semaphores instead of the Tile scheduler, which drops the
tile prologue/epilogue (pool memsets, semaphore range clear, per-engine
end barriers) from the critical path.

See kernel.py (v3) docstring for the host-side precompute contract.
"""

import numpy as np

B, C, H, W = 32, 7, 400, 400
T = 64
NCORES = 8
BLOC = B // NCORES          # 4 images per core
HW = H * W                  # 160000
CHW = C * HW                # 1120000
NELEM = BLOC * CHW          # 4480000 elements per core
PAD = 8                     # zero padding so 8-elem descriptors stay in bounds
NG = BLOC * T // 128        # 2 groups of 128 targets
P = 128
SLOT = 8                    # descriptor width in elements (32B aligned)
GC = NG * SLOT              # 16 value columns

_cached = {}
TRACE = False


def _build_nc():
    from contextlib import ExitStack

    import concourse.bacc as bacc
    import concourse.bass as bass
    import concourse.mybir as mybir

    f32 = mybir.dt.float32
    i32 = mybir.dt.int32
    OP = mybir.AluOpType
    AX = mybir.AxisListType

    nc = bacc.Bacc(
        "TRN2",
        target_bir_lowering=False,
        debug=False,
        enable_asserts=False,
        num_devices=NCORES,
    )

    preds_flat = nc.dram_tensor(
        "preds_flat", [NELEM + PAD, 1], f32, kind="ExternalInput"
    )
    aux_i = nc.dram_tensor("aux_i", [P, NG], i32, kind="ExternalInput")
    aux_f = nc.dram_tensor("aux_f", [P, 2 * GC], f32, kind="ExternalInput")
    out_d = nc.dram_tensor("out", [1, 1], f32, kind="ExternalOutput")

    with ExitStack() as ctx:
        ec = ctx.enter_context
        oi = ec(nc.sbuf_tensor([P, NG], i32))
        xf = ec(nc.sbuf_tensor([P, 2 * GC], f32))
        gat = ec(nc.sbuf_tensor([P, GC], f32))
        d = ec(nc.sbuf_tensor([P, GC], f32))
        ad = ec(nc.sbuf_tensor([P, GC], f32))
        mw = ec(nc.sbuf_tensor([P, GC], f32))
        su = ec(nc.sbuf_tensor([P, GC], f32))
        le = ec(nc.sbuf_tensor([P, GC], f32))
        ones = ec(nc.sbuf_tensor([P, 1], f32))
        red = ec(nc.sbuf_tensor([1, 1], f32))
        ps = ec(nc.psum_tensor([1, GC], f32))

        s_ai = ec(nc.semaphore())
        s_af = ec(nc.semaphore())
        s_gat = ec(nc.semaphore())
        s_v = ec(nc.semaphore())   # DVE chain counting sem (Tile-style)
        s_mm = ec(nc.semaphore())
        s_out = ec(nc.semaphore())

        with nc.Block(no_gpsimd_drain=True) as block:

            @block.sync
            def _(sync):
                sync.dma_start(oi[:, :], aux_i[:, :]).then_inc(s_ai, 16)
                sync.dma_start(xf[:, :], aux_f[:, :]).then_inc(s_af, 16)
                sync.wait_ge(s_v, 7)
                sync.dma_start(out_d[:, :], red[:, :]).then_inc(s_out, 16)
                sync.wait_ge(s_out, 16)

            @block.gpsimd
            def _(g):
                g.wait_ge(s_ai, 16)
                g.indirect_dma_start(
                    out=gat[:, :],
                    out_offset=None,
                    in_=preds_flat[:, :],
                    in_offset=bass.IndirectOffsetOnAxis(ap=oi[:, :], axis=0),
                ).then_inc(s_gat, 16)

            @block.vector
            def _(v):
                v.memset(ones[:, :], 1.0).then_inc(s_v, 1)          # s_v=1
                v.wait_ge(s_af, 16)
                v.wait_ge(s_gat, 16)
                v.tensor_sub(d[:, :], gat[:, :], xf[:, 0:GC]).then_inc(s_v, 1)
                v.wait_ge(s_v, 2)
                v.scalar_tensor_tensor(
                    ad[:, :], d[:, :], -1.0, d[:, :], OP.mult, OP.max
                ).then_inc(s_v, 1)
                v.wait_ge(s_v, 3)
                v.tensor_tensor(
                    mw[:, :], ad[:, :], xf[:, GC : 2 * GC], OP.min
                ).then_inc(s_v, 1)
                v.wait_ge(s_v, 4)
                v.scalar_tensor_tensor(
                    su[:, :], mw[:, :], -0.5, ad[:, :], OP.mult, OP.add
                ).then_inc(s_v, 1)
                v.wait_ge(s_v, 5)
                v.tensor_mul(le[:, :], mw[:, :], su[:, :]).then_inc(s_v, 1)
                v.wait_ge(s_mm, 1)
                v.reduce_sum(red[:, :], ps[:, :], axis=AX.X).then_inc(s_v, 1)

            @block.tensor
            def _(t):
                t.wait_ge(s_v, 6)
                nc.tensor.matmul(
                    ps[:, :], ones[:, :], le[:, :], start=True, stop=True
                ).then_inc(s_mm, 1)

    nc.compile()
    return nc


def _get_nc():
    if "nc" not in _cached:
        _cached["nc"] = _build_nc()
    return _cached["nc"]


def _host_prep(targets):
    """Grid cells, last-writer-wins mask, and num_objects from targets only."""
    gx = np.clip(np.floor(targets[:, :, 0] * np.float32(5.0)), 0, W - 1).astype(
        np.int64
    )
    gy = np.clip(np.floor(targets[:, :, 1] * np.float32(5.0)), 0, H - 1).astype(
        np.int64
    )
    cell = gy * W + gx  # [B,T]
    win = np.zeros((B, T), np.float32)
    for b in range(B):
        last = {}
        for t in range(T):
            last[cell[b, t]] = t
        for t in last.values():
            win[b, t] = 1.0
    num = float(win.sum())
    return cell, win, num


def _regroup(x):
    """[4,T] per-image array -> [128,2] (partition p, group g) layout."""
    return np.ascontiguousarray(
        x.reshape(NG, 2, T).transpose(1, 2, 0).reshape(P, NG)
    )


def _make_in_maps(preds, targets):
    cell, win, num = _host_prep(targets)
    preds_t = np.ascontiguousarray(preds.transpose(0, 2, 3, 1))
    jbase = (np.arange(BLOC, dtype=np.int64) * CHW)[:, None]  # [4,1]
    zpad = np.zeros((PAD, 1), np.float32)

    in_maps = []
    for k in range(NCORES):
        pshard = np.concatenate(
            [preds_t[k * BLOC : (k + 1) * BLOC].reshape(NELEM, 1), zpad]
        )
        tshard = targets[k * BLOC : (k + 1) * BLOC]  # [4, 64, 7]
        tpad = np.zeros((BLOC, T, SLOT), np.float32)
        tpad[:, :, :C] = tshard
        tvp = tpad.reshape(NG, 2, T, SLOT).transpose(1, 2, 0, 3).reshape(P, GC)
        wpg = _regroup(win[k * BLOC : (k + 1) * BLOC])  # [128,2]
        winxp = np.zeros((P, GC), np.float32)
        winxp[:, 0:C] = wpg[:, 0:1]
        winxp[:, SLOT : SLOT + C] = wpg[:, 1:2]
        offs = _regroup(cell[k * BLOC : (k + 1) * BLOC] * C + jbase).astype(np.int32)
        aux_f = np.ascontiguousarray(np.hstack([tvp, winxp]).astype(np.float32))
        in_maps.append({"preds_flat": pshard, "aux_i": offs, "aux_f": aux_f})
    return in_maps, num


def kernel(preds, targets):
    from concourse.bass_utils import run_bass_kernel_spmd

    preds = np.ascontiguousarray(np.asarray(preds), dtype=np.float32)
    targets = np.ascontiguousarray(np.asarray(targets), dtype=np.float32)
    assert preds.shape == (B, C, H, W) and targets.shape == (B, T, C)

    nc = _get_nc()
    in_maps, num = _make_in_maps(preds, targets)
    res = run_bass_kernel_spmd(nc, in_maps, list(range(NCORES)), trace=TRACE)
    _cached["last_results"] = res

    lsum = np.float32(0.0)
    for k in range(NCORES):
        lsum = np.float32(lsum + np.float32(res.results[k]["out"].reshape(1)[0]))
    loss = np.float32(lsum / np.float32(np.float32(num) + np.float32(1e-6)))
    return loss, np.float32(num)


# revision 7
# speedup vs baseline: 1.3968x; 1.0522x over previous
"""Detection-loss kernel for Trainium2 — raw Bass (no TileContext).

Same algorithm as v3 (one 256-descriptor indirect gather of 32B slots,
5-op DVE smoothl1 chain with the collision mask folded in via
mw = min(|d|, win), ones-matmul partition reduce, 4B output) but with
hand-placed semaphores instead of the Tile scheduler, which drops the
tile prologue/epilogue (pool memsets, semaphore range clear, per-engine
end barriers) from the critical path.

See kernel.py (v3) docstring for the host-side precompute contract.
"""

import numpy as np

B, C, H, W = 32, 7, 400, 400
T = 64
NCORES = 8
BLOC = B // NCORES          # 4 images per core
HW = H * W                  # 160000
CHW = C * HW                # 1120000
NELEM = BLOC * CHW          # 4480000 elements per core
PAD = 8                     # zero padding so 8-elem descriptors stay in bounds
NG = BLOC * T // 128        # 2 groups of 128 targets
P = 128
SLOT = 8                    # descriptor width in elements (32B aligned)
GC = NG * SLOT              # 16 value columns

_cached = {}
TRACE = False


def _build_nc():
    from contextlib import ExitStack

    import concourse.bacc as bacc
    import concourse.bass as bass
    import concourse.mybir as mybir

    f32 = mybir.dt.float32
    i32 = mybir.dt.int32
    OP = mybir.AluOpType
    AX = mybir.AxisListType

    nc = bacc.Bacc(
        "TRN2",
        target_bir_lowering=False,
        debug=False,
        enable_asserts=False,
        num_devices=NCORES,
    )

    preds_flat = nc.dram_tensor(
        "preds_flat", [NELEM + PAD, 1], f32, kind="ExternalInput"
    )
    aux_i = nc.dram_tensor("aux_i", [P, NG], i32, kind="ExternalInput")
    aux_f = nc.dram_tensor("aux_f", [P, 2 * GC], f32, kind="ExternalInput")
    out_d = nc.dram_tensor("out", [1, 1], f32, kind="ExternalOutput")

    with ExitStack() as ctx:
        ec = ctx.enter_context
        oi = ec(nc.sbuf_tensor([P, NG], i32))
        xf = ec(nc.sbuf_tensor([P, 2 * GC], f32))
        gat = ec(nc.sbuf_tensor([P, GC], f32))
        d = ec(nc.sbuf_tensor([P, GC], f32))
        ad = ec(nc.sbuf_tensor([P, GC], f32))
        mw = ec(nc.sbuf_tensor([P, GC], f32))
        su = ec(nc.sbuf_tensor([P, GC], f32))
        le = ec(nc.sbuf_tensor([P, GC], f32))
        ones = ec(nc.sbuf_tensor([P, 1], f32))
        red = ec(nc.sbuf_tensor([1, 1], f32))
        ps = ec(nc.psum_tensor([1, GC], f32))

        s_ai = ec(nc.semaphore())
        s_af = ec(nc.semaphore())
        s_gat = ec(nc.semaphore())
        s_v = ec(nc.semaphore())   # DVE chain counting sem (Tile-style)
        s_mm = ec(nc.semaphore())
        s_out = ec(nc.semaphore())

        with nc.Block() as block:

            @block.sync
            def _(sync):
                sync.dma_start(oi[:, :], aux_i[:, :]).then_inc(s_ai, 16)
                sync.dma_start(xf[:, :], aux_f[:, :]).then_inc(s_af, 16)
                sync.wait_ge(s_v, 7)
                # completion (s_out) is covered by the block-end engine
                # drains + runtime teardown; no explicit wait needed
                sync.dma_start(out_d[:, :], red[:, :]).then_inc(s_out, 16)

            @block.gpsimd
            def _(g):
                g.wait_ge(s_ai, 16)
                g.indirect_dma_start(
                    out=gat[:, :],
                    out_offset=None,
                    in_=preds_flat[:, :],
                    in_offset=bass.IndirectOffsetOnAxis(ap=oi[:, :], axis=0),
                ).then_inc(s_gat, 16)

            @block.vector
            def _(v):
                v.memset(ones[:, :], 1.0).then_inc(s_v, 1)          # s_v=1
                v.wait_ge(s_af, 16)
                v.wait_ge(s_gat, 16)
                v.tensor_sub(d[:, :], gat[:, :], xf[:, 0:GC]).then_inc(s_v, 1)
                v.wait_ge(s_v, 2)
                v.scalar_tensor_tensor(
                    ad[:, :], d[:, :], -1.0, d[:, :], OP.mult, OP.max
                ).then_inc(s_v, 1)
                v.wait_ge(s_v, 3)
                v.tensor_tensor(
                    mw[:, :], ad[:, :], xf[:, GC : 2 * GC], OP.min
                ).then_inc(s_v, 1)
                v.wait_ge(s_v, 4)
                v.scalar_tensor_tensor(
                    su[:, :], mw[:, :], -0.5, ad[:, :], OP.mult, OP.add
                ).then_inc(s_v, 1)
                v.wait_ge(s_v, 5)
                v.tensor_mul(le[:, :], mw[:, :], su[:, :]).then_inc(s_v, 1)
                v.wait_ge(s_mm, 1)
                v.reduce_sum(red[:, :], ps[:, :], axis=AX.X).then_inc(s_v, 1)

            @block.tensor
            def _(t):
                t.wait_ge(s_v, 6)
                nc.tensor.matmul(
                    ps[:, :], ones[:, :], le[:, :], start=True, stop=True
                ).then_inc(s_mm, 1)

    nc.compile()
    return nc


def _get_nc():
    if "nc" not in _cached:
        _cached["nc"] = _build_nc()
    return _cached["nc"]


def _host_prep(targets):
    """Grid cells, last-writer-wins mask, and num_objects from targets only."""
    gx = np.clip(np.floor(targets[:, :, 0] * np.float32(5.0)), 0, W - 1).astype(
        np.int64
    )
    gy = np.clip(np.floor(targets[:, :, 1] * np.float32(5.0)), 0, H - 1).astype(
        np.int64
    )
    cell = gy * W + gx  # [B,T]
    win = np.zeros((B, T), np.float32)
    for b in range(B):
        last = {}
        for t in range(T):
            last[cell[b, t]] = t
        for t in last.values():
            win[b, t] = 1.0
    num = float(win.sum())
    return cell, win, num


def _regroup(x):
    """[4,T] per-image array -> [128,2] (partition p, group g) layout."""
    return np.ascontiguousarray(
        x.reshape(NG, 2, T).transpose(1, 2, 0).reshape(P, NG)
    )


def _make_in_maps(preds, targets):
    cell, win, num = _host_prep(targets)
    preds_t = np.ascontiguousarray(preds.transpose(0, 2, 3, 1))
    jbase = (np.arange(BLOC, dtype=np.int64) * CHW)[:, None]  # [4,1]
    zpad = np.zeros((PAD, 1), np.float32)

    in_maps = []
    for k in range(NCORES):
        pshard = np.concatenate(
            [preds_t[k * BLOC : (k + 1) * BLOC].reshape(NELEM, 1), zpad]
        )
        tshard = targets[k * BLOC : (k + 1) * BLOC]  # [4, 64, 7]
        tpad = np.zeros((BLOC, T, SLOT), np.float32)
        tpad[:, :, :C] = tshard
        tvp = tpad.reshape(NG, 2, T, SLOT).transpose(1, 2, 0, 3).reshape(P, GC)
        wpg = _regroup(win[k * BLOC : (k + 1) * BLOC])  # [128,2]
        winxp = np.zeros((P, GC), np.float32)
        winxp[:, 0:C] = wpg[:, 0:1]
        winxp[:, SLOT : SLOT + C] = wpg[:, 1:2]
        offs = _regroup(cell[k * BLOC : (k + 1) * BLOC] * C + jbase).astype(np.int32)
        aux_f = np.ascontiguousarray(np.hstack([tvp, winxp]).astype(np.float32))
        in_maps.append({"preds_flat": pshard, "aux_i": offs, "aux_f": aux_f})
    return in_maps, num


def kernel(preds, targets):
    from concourse.bass_utils import run_bass_kernel_spmd

    preds = np.ascontiguousarray(np.asarray(preds), dtype=np.float32)
    targets = np.ascontiguousarray(np.asarray(targets), dtype=np.float32)
    assert preds.shape == (B, C, H, W) and targets.shape == (B, T, C)

    nc = _get_nc()
    in_maps, num = _make_in_maps(preds, targets)
    res = run_bass_kernel_spmd(nc, in_maps, list(range(NCORES)), trace=TRACE)
    _cached["last_results"] = res

    lsum = np.float32(0.0)
    for k in range(NCORES):
        lsum = np.float32(lsum + np.float32(res.results[k]["out"].reshape(1)[0]))
    loss = np.float32(lsum / np.float32(np.float32(num) + np.float32(1e-6)))
    return loss, np.float32(num)
